# revision 1
# baseline (speedup 1.0000x reference)
"""BipartiteSAGEConv on 8 Trainium2 NeuronCores.

out = normalize(mean_{dst}(x[src]) @ W_l + b_l + x @ W_r)

Strategy:
- Host: sort edges by destination node, shard destination-node ranges across
  the 8 cores (each core owns 12500 contiguous nodes and all edges pointing
  into them -> no cross-core reduction needed). Per 128-node tile, edges are
  grouped by src bank (4 banks of 25024 rows, since dma_gather indices are
  int16) and packed into KB chunks of 128 per bank (padded; padding edges
  carry dstrel=-1 so the one-hot kills them). The per-edge weight
  w = 1/max(deg(dst),1) is folded into the one-hot so the PSUM accumulation
  yields the mean directly.
- Device (SPMD, identical program on all 8 cores):
  * dma_gather (Ant SWDGE gather, int16 idx) of x[src] rows per (tile, bank)
  * DVE builds the weighted one-hot: (iota == dstrel) * w  (one fused op)
  * PE accumulates meanT[f, n] += msg[e, f].T @ onehot[e, n] over chunks
  * PE: out[n, fo] = meanT.T @ W_l + xT.T @ W_r + ones x b_l (one PSUM group)
  * ACT Square+accum -> row sum of squares; sqrt; clamp; DVE reciprocal;
    scale rows; DMA out.
"""

import numpy as np

N_NODES = 100000
D = 128
N_CORES = 8
NODES_PER_CORE = N_NODES // N_CORES  # 12500
P = 128
TILES_PER_CORE = (NODES_PER_CORE + P - 1) // P  # 98
NODE_PAD = TILES_PER_CORE * P  # 12544
X_PAD_ROWS = 100096  # 782 * 128; >= 7*12500 + 12544
BANK = X_PAD_ROWS // 4  # 25024 rows per gather bank (< 32768 int16 limit)
NBANKS = 4

_program_cache = {}

# test harness hooks
TRACE = False
LAST = {}

NQUEUES = 4  # SWDGE queues; gathers round-robin across them
SCRATCH = 16384  # SWDGE descriptor-ring carveout bytes (ring = SCRATCH // 16)
GATHER_BF16 = True  # gather from a bf16 copy of x (halves gather bytes);
SINGLE_PACKET = True  # dma_gather packet mode (HW ucode knob, A/B on HW)
# the one-hot is then exact 0/1 bf16 and the 1/deg scaling happens in f32
# on the (summed @ W_l) product instead of being folded into the one-hot.


def _build_program(KB: int, bench_repeat: int = 1, ablate: str = ""):
    """Build + compile the SPMD Bass program; KB = edge chunks per (tile, bank).

    bench_repeat > 1 wraps the main loop in a For_i that recomputes the same
    output bench_repeat times (for device-time measurement only).
    ablate: comma-set of {gather, onehot, chunkmm} to skip (bench only).
    """
    ablate_set = set(ablate.split(",")) if ablate else set()
    import contextlib

    import concourse.bass as bass
    import concourse.tile as tile
    from concourse import bacc, mybir
    from concourse.masks import make_identity

    f32 = mybir.dt.float32
    bf16 = mybir.dt.bfloat16
    i16 = mybir.dt.int16
    gdt = bf16 if GATHER_BF16 else f32
    KT = NBANKS * KB  # chunk slots per tile
    NIDX = KB * P  # indices per gather
    IW = NIDX // 16  # idx columns per (tile, bank)

    nc = bacc.Bacc(
        "TRN2",
        target_bir_lowering=False,
        debug=False,
        num_devices=N_CORES,
        num_swdge_queues=NQUEUES,
        dynamic_dma_scratch_size=SCRATCH,
    )

    if GATHER_BF16:
        xgat = nc.dram_tensor("xbf", [X_PAD_ROWS, D], bf16, kind="ExternalInput")
        rcol = nc.dram_tensor("rcol", [P, TILES_PER_CORE], f32, kind="ExternalInput")
    else:
        xgat = nc.dram_tensor("xpad", [X_PAD_ROWS, D], f32, kind="ExternalInput")
    xchunk = nc.dram_tensor("xchunk", [NODE_PAD, D], f32, kind="ExternalInput")
    gidx = nc.dram_tensor(
        "gidx", [P, TILES_PER_CORE, NBANKS, IW], i16, kind="ExternalInput"
    )
    dstrel = nc.dram_tensor("dstrel", [P, TILES_PER_CORE, KT], f32, kind="ExternalInput")
    wgt = nc.dram_tensor("wgt", [P, TILES_PER_CORE, KT], f32, kind="ExternalInput")
    wl = nc.dram_tensor("wl", [D, D], f32, kind="ExternalInput")
    wr = nc.dram_tensor("wr", [D, D], f32, kind="ExternalInput")
    bl = nc.dram_tensor("bl", [1, D], f32, kind="ExternalInput")
    out = nc.dram_tensor("out", [NODE_PAD, D], f32, kind="ExternalOutput")

    with tile.TileContext(nc) as tc:
        with (
            tc.tile_pool(name="const", bufs=1) as const_pool,
            tc.tile_pool(name="meta", bufs=1) as meta_pool,
            tc.tile_pool(name="msg", bufs=3 * NBANKS) as msg_pool,
            tc.tile_pool(name="oh", bufs=6) as oh_pool,
            tc.tile_pool(name="xt", bufs=3) as xt_pool,
            tc.tile_pool(name="ep", bufs=3) as ep_pool,
            tc.tile_pool(name="ps_mean", bufs=2, space="PSUM") as ps_mean_pool,
            tc.tile_pool(name="ps_xt", bufs=2, space="PSUM") as ps_xt_pool,
            tc.tile_pool(name="ps_out", bufs=2, space="PSUM") as ps_out_pool,
        ):
            # ---- constants / weights / metadata (loaded once) ----
            iota_i = const_pool.tile([P, P], mybir.dt.int32)
            nc.gpsimd.iota(iota_i[:], pattern=[[1, P]], base=0, channel_multiplier=0)
            iota_f = const_pool.tile([P, P], f32)
            nc.vector.tensor_copy(iota_f[:], iota_i[:])
            if GATHER_BF16:
                iota_g = const_pool.tile([P, P], bf16)
                nc.vector.tensor_copy(iota_g[:], iota_i[:])
                rcol_sb = const_pool.tile([P, TILES_PER_CORE], f32)
                nc.sync.dma_start(rcol_sb[:], rcol[:])
            else:
                iota_g = iota_f

            identity = const_pool.tile([P, P], f32)
            make_identity(nc, identity[:])

            wl_sb = const_pool.tile([D, D], f32)
            nc.sync.dma_start(wl_sb[:], wl[:])
            wr_sb = const_pool.tile([D, D], f32)
            nc.sync.dma_start(wr_sb[:], wr[:])
            bl_sb = const_pool.tile([1, D], f32)
            nc.sync.dma_start(bl_sb[:], bl[:])
            ones1 = const_pool.tile([1, D], f32)
            nc.vector.memset(ones1[:], 1.0)

            idx_all = meta_pool.tile([P, TILES_PER_CORE, NBANKS, IW], i16)
            nc.sync.dma_start(idx_all[:], gidx[:])
            dst_all = meta_pool.tile([P, TILES_PER_CORE, KT], f32)
            nc.sync.dma_start(dst_all[:], dstrel[:])
            w_all = meta_pool.tile([P, TILES_PER_CORE, KT], f32)
            nc.sync.dma_start(w_all[:], wgt[:])

            # ---- main loop over node tiles ----
            rep_ctx = (
                tc.For_i(0, bench_repeat, 1)
                if bench_repeat > 1
                else contextlib.nullcontext()
            )
            with rep_ctx:
              for t in range(TILES_PER_CORE):
                # gather x[src] per bank: msg position (p, j) <- edge j*128+p
                msgs = []
                if "gather" not in ablate_set:
                    nb = 2 if "2banks" in ablate_set else NBANKS
                    elem = D // 2 if "half" in ablate_set else D
                    estep = D if "half" in ablate_set else None
                    for b in range(nb):
                        xpad_v = xgat[b * BANK : (b + 1) * BANK, :elem]
                        msg = msg_pool.tile([P, KB, elem], gdt, tag="msg")
                        if "splitgather" in ablate_set:
                            k1 = 2  # chunks in first gather
                            nc.gpsimd.dma_gather(
                                out_ap=msg[:, :k1, :],
                                in_ap=xpad_v,
                                idxs_ap=idx_all[:, t, b, : k1 * 8],
                                num_idxs=k1 * P,
                                num_idxs_reg=k1 * P,
                                elem_size=elem,
                                elem_step=estep,
                                queue_num=b % NQUEUES,
                            )
                            nc.gpsimd.dma_gather(
                                out_ap=msg[:, k1:, :],
                                in_ap=xpad_v,
                                idxs_ap=idx_all[:, t, b, k1 * 8 :],
                                num_idxs=(KB - k1) * P,
                                num_idxs_reg=(KB - k1) * P,
                                elem_size=elem,
                                elem_step=estep,
                                queue_num=b % NQUEUES,
                            )
                        else:
                            nc.gpsimd.dma_gather(
                                out_ap=msg[:],
                                in_ap=xpad_v,
                                idxs_ap=idx_all[:, t, b, :],
                                num_idxs=NIDX,
                                num_idxs_reg=NIDX,
                                elem_size=elem,
                                elem_step=estep,
                                single_packet=SINGLE_PACKET,
                                queue_num=b % NQUEUES,
                            )
                        msgs.append(msg)

                # root path: x tile, transposed via PE
                x_sb = xt_pool.tile([P, D], f32, tag="x_in")
                nc.sync.dma_start(x_sb[:], xchunk[t * P : (t + 1) * P, :])
                ps_xt = ps_xt_pool.tile([P, P], f32)
                nc.tensor.transpose(out=ps_xt[:], in_=x_sb[:], identity=identity[:])
                xT_sb = xt_pool.tile([P, D], f32, tag="x_t")
                nc.scalar.copy(xT_sb[:], ps_xt[:])

                # aggregation: sumT/meanT[f, n] accumulated over chunk slots
                ps_mean = ps_mean_pool.tile([P, P], f32)
                for s in range(KT):
                    b, j = divmod(s, KB)
                    if "onehot" not in ablate_set:
                        oh = oh_pool.tile([P, P], gdt)
                        if GATHER_BF16:
                            nc.vector.tensor_scalar(
                                oh[:],
                                iota_g[:],
                                dst_all[:, t, s : s + 1],
                                None,
                                mybir.AluOpType.is_equal,
                            )
                        else:
                            nc.vector.tensor_scalar(
                                oh[:],
                                iota_g[:],
                                dst_all[:, t, s : s + 1],
                                w_all[:, t, s : s + 1],
                                mybir.AluOpType.is_equal,
                                mybir.AluOpType.mult,
                            )
                        rhs_ap = oh[:]
                    else:
                        rhs_ap = iota_g[:]
                    if "chunkmm" not in ablate_set:
                        lhs_ap = (
                            msgs[b][:, j, :]
                            if "gather" not in ablate_set
                            else iota_g[:]
                        )
                        nc.tensor.matmul(
                            out=ps_mean[:],
                            lhsT=lhs_ap,
                            rhs=rhs_ap,
                            start=(s == 0),
                            stop=(s == KT - 1),
                        )
                if "chunkmm" in ablate_set:
                    nc.tensor.matmul(
                        out=ps_mean[:],
                        lhsT=iota_g[:],
                        rhs=iota_g[:],
                        start=True,
                        stop=True,
                    )
                meanT_sb = ep_pool.tile([P, P], f32, tag="meanT")
                nc.scalar.copy(meanT_sb[:], ps_mean[:])

                if GATHER_BF16:
                    # ps_a = sumT.T @ W_l; scale rows by 1/deg (exact f32)
                    ps_a = ps_out_pool.tile([P, P], f32, tag="ps_a")
                    nc.tensor.matmul(
                        out=ps_a[:], lhsT=meanT_sb[:], rhs=wl_sb[:],
                        start=True, stop=True,
                    )
                    out_l = ep_pool.tile([P, P], f32, tag="out_l")
                    nc.vector.tensor_scalar(
                        out_l[:],
                        ps_a[:],
                        rcol_sb[:, t : t + 1],
                        None,
                        mybir.AluOpType.mult,
                    )
                    # ps_b = xT.T @ W_r + ones x b_l; final = out_l + ps_b
                    ps_o = ps_out_pool.tile([P, P], f32, tag="ps_b")
                    nc.tensor.matmul(
                        out=ps_o[:], lhsT=xT_sb[:], rhs=wr_sb[:],
                        start=True, stop=False,
                    )
                    nc.tensor.matmul(
                        out=ps_o[:], lhsT=ones1[:], rhs=bl_sb[:],
                        start=False, stop=True,
                    )
                    final = ep_pool.tile([P, P], f32, tag="final")
                    nc.vector.tensor_tensor(
                        out=final[:], in0=out_l[:], in1=ps_o[:],
                        op=mybir.AluOpType.add,
                    )
                    norm_src = final[:]
                else:
                    # linear: out[n,fo] = meanT.T @ W_l + xT.T @ W_r + ones x b_l
                    ps_o = ps_out_pool.tile([P, P], f32, tag="ps_b")
                    nc.tensor.matmul(
                        out=ps_o[:], lhsT=meanT_sb[:], rhs=wl_sb[:],
                        start=True, stop=False,
                    )
                    nc.tensor.matmul(
                        out=ps_o[:], lhsT=xT_sb[:], rhs=wr_sb[:],
                        start=False, stop=False,
                    )
                    nc.tensor.matmul(
                        out=ps_o[:], lhsT=ones1[:], rhs=bl_sb[:],
                        start=False, stop=True,
                    )
                    norm_src = ps_o[:]

                # row-wise L2 normalize: out / max(||out||, 1e-12)
                sq_scr = ep_pool.tile([P, P], f32, tag="sq")
                ss = ep_pool.tile([P, 1], f32, tag="ss")
                nc.scalar.activation(
                    sq_scr[:],
                    norm_src,
                    mybir.ActivationFunctionType.Square,
                    accum_out=ss[:],
                )
                nrm = ep_pool.tile([P, 1], f32, tag="nrm")
                nc.scalar.sqrt(nrm[:], ss[:])
                nrmc = ep_pool.tile([P, 1], f32, tag="nrmc")
                nc.vector.tensor_scalar_max(nrmc[:], nrm[:], 1e-12)
                rn = ep_pool.tile([P, 1], f32, tag="rn")
                nc.vector.reciprocal(rn[:], nrmc[:])

                out_sb = ep_pool.tile([P, P], f32, tag="out")
                nc.vector.tensor_scalar(
                    out_sb[:],
                    norm_src,
                    rn[:, :1],
                    None,
                    mybir.AluOpType.mult,
                )
                nc.sync.dma_start(out[t * P : (t + 1) * P, :], out_sb[:])

    nc.compile()
    return nc


def _prepare(x, edge_index):
    """Host-side sharding: sort by dst, group per (tile, bank), pack chunks."""
    src = np.ascontiguousarray(edge_index[0]).astype(np.int64)
    dst = np.ascontiguousarray(edge_index[1]).astype(np.int64)

    cnt = np.bincount(dst, minlength=N_NODES)
    w_node = (1.0 / np.maximum(cnt, 1)).astype(np.float32)

    order = np.argsort(dst, kind="stable")
    src_s = src[order]
    dst_s = dst[order]

    # per-core edge ranges and per (core,tile,bank) grouping
    per_core = []
    KB = 1
    for c in range(N_CORES):
        base = c * NODES_PER_CORE
        lo = np.searchsorted(dst_s, base)
        hi = np.searchsorted(dst_s, base + NODES_PER_CORE)
        s_c = src_s[lo:hi]
        d_c = dst_s[lo:hi] - base
        t_c = d_c // P
        b_c = s_c // BANK
        key = (t_c * NBANKS + b_c).astype(np.int64)
        ordc = np.argsort(key, kind="stable")
        s_c, d_c, key = s_c[ordc], d_c[ordc], key[ordc]
        counts = np.bincount(key, minlength=TILES_PER_CORE * NBANKS)
        KB = max(KB, int(np.ceil(counts.max() / P)))
        per_core.append((s_c, d_c, counts))

    KT = NBANKS * KB
    NIDX = KB * P
    IW = NIDX // 16

    # per-node 1/max(deg,1) as [core][lane, tile] columns
    wg = np.ones(X_PAD_ROWS, np.float32)
    wg[:N_NODES] = w_node
    rcol = np.zeros((N_CORES, P, TILES_PER_CORE), np.float32)
    for c in range(N_CORES):
        idx = (
            c * NODES_PER_CORE
            + (np.arange(TILES_PER_CORE) * P)[None, :]
            + np.arange(P)[:, None]
        )
        rcol[c] = wg[idx]

    gidx = np.zeros((N_CORES, P, TILES_PER_CORE, NBANKS, IW), np.int16)
    dstrel = np.full((N_CORES, P, TILES_PER_CORE, KT), -1.0, np.float32)
    wgt = np.zeros((N_CORES, P, TILES_PER_CORE, KT), np.float32)

    prow = np.arange(P) % 16
    scol = np.arange(IW) * 16
    for c in range(N_CORES):
        s_c, d_c, counts = per_core[c]
        starts = np.concatenate([[0], np.cumsum(counts)])
        for t in range(TILES_PER_CORE):
            for b in range(NBANKS):
                g = t * NBANKS + b
                n = counts[g]
                if n == 0:
                    continue
                lo = starts[g]
                sv = s_c[lo : lo + n] - b * BANK
                dv = (d_c[lo : lo + n] - t * P).astype(np.float32)
                wv = w_node[d_c[lo : lo + n] + c * NODES_PER_CORE]
                i_pad = np.zeros(NIDX, np.int16)
                i_pad[:n] = sv.astype(np.int16)
                d_pad = np.full(NIDX, -1.0, np.float32)
                d_pad[:n] = dv
                w_pad = np.zeros(NIDX, np.float32)
                w_pad[:n] = wv
                # idx position i lives at [i % 16, i // 16], replicated %16
                gidx[c, :, t, b, :] = i_pad[scol[None, :] + prow[:, None]]
                # chunk slot s=b*KB+j, lane p <- edge j*128+p
                dstrel[c, :, t, b * KB : (b + 1) * KB] = d_pad.reshape(KB, P).T
                wgt[c, :, t, b * KB : (b + 1) * KB] = w_pad.reshape(KB, P).T

    return gidx, dstrel, wgt, rcol, KB


def kernel(x, edge_index, W_l, b_l, W_r):
    from concourse.bass_utils import run_bass_kernel_spmd

    x = np.ascontiguousarray(np.asarray(x, dtype=np.float32))
    W_l = np.ascontiguousarray(np.asarray(W_l, dtype=np.float32))
    W_r = np.ascontiguousarray(np.asarray(W_r, dtype=np.float32))
    b_l = np.ascontiguousarray(np.asarray(b_l, dtype=np.float32)).reshape(1, D)

    gidx, dstrel, wgt, rcol, KB = _prepare(x, np.asarray(edge_index))

    xpad = np.zeros((X_PAD_ROWS, D), np.float32)
    xpad[:N_NODES] = x
    if GATHER_BF16:
        import ml_dtypes

        xbf = xpad.astype(ml_dtypes.bfloat16)

    if KB not in _program_cache:
        _program_cache[KB] = _build_program(KB)
    nc = _program_cache[KB]

    in_maps = []
    for c in range(N_CORES):
        base = c * NODES_PER_CORE
        m = {
            "xchunk": xpad[base : base + NODE_PAD],
            "gidx": gidx[c],
            "dstrel": dstrel[c],
            "wgt": wgt[c],
            "wl": W_l,
            "wr": W_r,
            "bl": b_l,
        }
        if GATHER_BF16:
            m["xbf"] = xbf
            m["rcol"] = rcol[c]
        else:
            m["xpad"] = xpad
        in_maps.append(m)

    LAST["nc"] = nc
    LAST["in_maps"] = in_maps
    r = run_bass_kernel_spmd(nc, in_maps, list(range(N_CORES)), trace=TRACE)
    LAST["exec_time_ns"] = r.exec_time_ns
    res = r.results
    out = np.concatenate(
        [res[c]["out"][:NODES_PER_CORE] for c in range(N_CORES)], axis=0
    )
    return out



# revision 27
# speedup vs baseline: 109.2144x; 109.2144x over previous
"""BipartiteSAGEConv on 8 Trainium2 NeuronCores.

out = normalize(mean_{dst}(x[src]) @ W_l + b_l + x @ W_r)

Strategy:
- Host: sort edges by destination node, shard destination-node ranges across
  the 8 cores (each core owns 12500 contiguous nodes and all edges pointing
  into them -> no cross-core reduction needed). Per 128-node tile, edges are
  grouped by src bank (4 banks of 25024 rows, since dma_gather indices are
  int16) and packed into KB chunks of 128 per bank (padded; padding edges
  carry dstrel=-1 so the one-hot kills them). The host also provides a
  per-core transposed x (xT) so the root path needs no on-device transpose,
  and per-node 1/max(deg,1) columns (rcol) for the mean scaling.
- Device (SPMD, identical program on all 8 cores), per group of G=7 tiles:
  * 4 dma_gather ops (one per src bank, G tiles batched per gather to
    amortize the ~1us SWDGE descriptor-generation fixed cost; the gather is
    descriptor-RATE-bound, not bandwidth-bound, so bf16 256B elements) of
    x[src] rows
  * one HWDGE load of the group's xT tiles, one store of the group's out
  * per tile: ONE batched DVE tensor_tensor is_equal over [128, KT*128]
    builds all chunk one-hots at once (tensor_tensor never enters the DVE
    2-port perf mode, so unlike tensor_scalar it does not lock GPSIMD's
    SWDGE descriptor generation out of SBUF -- this is what lets the
    gathers overlap the compute); PE accumulates
    sumT[f,n] += msg[e,f].T @ onehot[e,n] over chunk slots; ACT copies
    sumT to SBUF; PE: ps_a = sumT.T @ W_l, DVE scales rows by 1/deg (f32,
    exact); PE: ps_b = xT.T @ W_r + ones x b_l; DVE adds; ACT Square+accum
    -> row sum of squares; ACT Sqrt(+1e-24); DVE reciprocal; DVE scales
    rows into the group out tile.

Known-good HW footguns encoded here:
- single_packet=True dma_gather breaks on HW above ~640 idxs; grouped
  gathers use single_packet=False.
- num_idxs_reg must equal the count of non-negative indices; trailing -1
  index padding generates no descriptors (when per-tile sub-gathers are
  used; grouped gathers 0-pad instead since -1s must be strictly trailing).
- Never-gathered msg chunk slots must hold finite values (0 * NaN = NaN in
  the PE accumulation), hence the explicit ping-pong msg buffers zeroed
  once at startup.
- Tile-major gather issue order keeps Tile's 8-lane DMASW semaphore
  round-robin aligned with the 4 SWDGE queues (a sem lane must not be
  shared across queues).
"""

import numpy as np

N_NODES = 100000
D = 128
N_CORES = 8
NODES_PER_CORE = N_NODES // N_CORES  # 12500
P = 128
TILES_PER_CORE = (NODES_PER_CORE + P - 1) // P  # 98
NODE_PAD = TILES_PER_CORE * P  # 12544
GROUP = 7  # tiles per gather/DMA group
NG = TILES_PER_CORE // GROUP  # 14
SINGLE_PACKET = True  # True breaks on HW for num_idxs > ~640 -> per-tile sub-gathers
GATHER_F32 = False
BIGGATHER = True  # one gather per (group, bank); single_packet off (capped ~640 idxs)


def set_group(g):
    """Set tiles-per-group (must divide 98). For HW bring-up experiments."""
    global GROUP, NG
    assert TILES_PER_CORE % g == 0
    GROUP = g
    NG = TILES_PER_CORE // g
    _program_cache.clear()
X_PAD_ROWS = 100096  # 782 * 128; >= 7*12500 + 12544
BANK = X_PAD_ROWS // 4  # 25024 rows per gather bank (< 32768 int16 limit)
NBANKS = 4
NQUEUES = 4  # SWDGE queues; banks round-robin across them
SCRATCH = 32768  # SWDGE descriptor-ring carveout bytes (ring = SCRATCH//16 descs)

_program_cache = {}

# test harness hooks
TRACE = False
LAST = {}


def _build_program(KB: int, bench_repeat: int = 1, ablate: str = ""):
    """Build + compile the SPMD Bass program; KB = edge chunks per (tile, bank).

    bench_repeat > 1 wraps the main loop in a For_i that recomputes the same
    output bench_repeat times (for device-time measurement only).
    ablate: comma-set of {gather, onehot, chunkmm} to skip (bench only).
    """
    ablate_set = set(ablate.split(",")) if ablate else set()
    import contextlib

    import concourse.bass as bass
    import concourse.tile as tile
    from concourse import bacc, mybir
    from concourse.masks import make_identity

    f32 = mybir.dt.float32
    bf16 = mybir.dt.bfloat16
    i16 = mybir.dt.int16
    KT = NBANKS * KB  # chunk slots per tile
    GIDX = GROUP * KB * P  # indices per gather (one bank, G tiles)
    GIW = GIDX // 16  # idx columns per (group, bank)

    nc = bacc.Bacc(
        "TRN2",
        target_bir_lowering=False,
        debug=False,
        num_devices=N_CORES,
        num_swdge_queues=NQUEUES,
        dynamic_dma_scratch_size=SCRATCH,
    )

    gdt = f32 if GATHER_F32 else bf16
    xbf = nc.dram_tensor("xbf", [X_PAD_ROWS, D], gdt, kind="ExternalInput")
    xt = nc.dram_tensor("xt", [NG, P, GROUP * D], f32, kind="ExternalInput")
    gidx = nc.dram_tensor("gidx", [P, NG, NBANKS, GIW], i16, kind="ExternalInput")
    cnt = nc.dram_tensor(
        "cnt", [1, NG * NBANKS * GROUP], mybir.dt.int32, kind="ExternalInput"
    )
    dstrel = nc.dram_tensor("dstrel", [P, TILES_PER_CORE, KT], bf16, kind="ExternalInput")
    rcol = nc.dram_tensor("rcol", [P, TILES_PER_CORE], f32, kind="ExternalInput")
    wl = nc.dram_tensor("wl", [D, D], f32, kind="ExternalInput")
    wr = nc.dram_tensor("wr", [D, D], f32, kind="ExternalInput")
    bl = nc.dram_tensor("bl", [1, D], f32, kind="ExternalInput")
    out = nc.dram_tensor("out", [NG, P, GROUP * D], f32, kind="ExternalOutput")

    with tile.TileContext(nc) as tc:
        with (
            tc.tile_pool(name="const", bufs=1) as const_pool,
            tc.tile_pool(name="meta", bufs=1) as meta_pool,
            tc.tile_pool(name="msg", bufs=1) as msg_pool,
            tc.tile_pool(name="oh", bufs=4) as oh_pool,
            tc.tile_pool(name="xt", bufs=2) as xt_pool,
            tc.tile_pool(name="mean", bufs=3) as mean_pool,
            tc.tile_pool(name="eg", bufs=4) as eg_pool,
            tc.tile_pool(name="outp", bufs=3) as out_pool,
            tc.tile_pool(name="ps_sum", bufs=3, space="PSUM") as ps_sum_pool,
            tc.tile_pool(name="ps_out", bufs=2, space="PSUM") as ps_out_pool,
        ):
            # ---- constants / weights / metadata (loaded once) ----
            iota_i = const_pool.tile([P, P], mybir.dt.int32)
            nc.gpsimd.iota(iota_i[:], pattern=[[1, P]], base=0, channel_multiplier=0)
            iota_g = const_pool.tile([P, P], bf16)
            nc.vector.tensor_copy(iota_g[:], iota_i[:])

            iota_rep = const_pool.tile([P, KT, P], bf16)
            for s in range(KT):
                nc.vector.tensor_copy(iota_rep[:, s, :], iota_g[:])

            identity = const_pool.tile([P, P], f32)
            make_identity(nc, identity[:])

            wl_sb = const_pool.tile([D, D], f32)
            nc.sync.dma_start(wl_sb[:], wl[:])
            wr_sb = const_pool.tile([D, D], f32)
            nc.sync.dma_start(wr_sb[:], wr[:])
            bl_sb = const_pool.tile([1, D], f32)
            nc.sync.dma_start(bl_sb[:], bl[:])
            ones1 = const_pool.tile([1, D], f32)
            nc.vector.memset(ones1[:], 1.0)
            eps = const_pool.tile([P, 1], f32)
            nc.vector.memset(eps[:], 1e-24)

            idx_all = meta_pool.tile([P, NG, NBANKS, GIW], i16)
            nc.sync.dma_start(idx_all[:], gidx[:])
            cnt_sb = meta_pool.tile([1, NG * NBANKS * GROUP], mybir.dt.int32)
            nc.sync.dma_start(cnt_sb[:], cnt[:])
            cnt_reg = nc.gpsimd.alloc_register("cnt_reg")

            # persistent ping-pong gather buffers (explicit double buffer).
            # Zeroed once: chunk slots whose gather is trimmed (trailing -1
            # idxs) must stay finite, since 0 * NaN = NaN in the
            # accumulation matmul.
            msg_slots = []
            for b in range(NBANKS):
                pair = []
                for h in range(2):
                    mslot = msg_pool.tile(
                        [P, GROUP * KB, D], gdt, tag=f"msg{b}_{h}"
                    )
                    nc.vector.memset(mslot[:], 0.0)
                    pair.append(mslot)
                msg_slots.append(pair)
            dst_all = meta_pool.tile([P, TILES_PER_CORE, KT], bf16)
            nc.sync.dma_start(dst_all[:], dstrel[:])
            rcol_sb = meta_pool.tile([P, TILES_PER_CORE], f32)
            nc.sync.dma_start(rcol_sb[:], rcol[:])

            # ---- main loop over groups of GROUP node tiles ----
            rep_ctx = (
                tc.For_i(0, bench_repeat, 1)
                if bench_repeat > 1
                else contextlib.nullcontext()
            )
            with rep_ctx:
              for g in range(NG):
                # gather x[src] per bank for the whole group:
                # msg position (p, j) <- edge j*128+p of the group's packing
                if "gather" not in ablate_set:
                    TIW = KB * P // 16  # idx cols per tile within the group
                    msgs = [msg_slots[b][g % 2] for b in range(NBANKS)]
                    # single-packet mode is faster per descriptor; the
                    # descriptor ring (SCRATCH//16 entries) bounds idxs per
                    # gather. num_idxs_reg = count of real (non-negative)
                    # idxs: trailing -1 padding generates no descriptors.
                    # Tile-major issue order keeps Tile's 8-lane DMASW
                    # round-robin aligned with the 4 SWDGE queues (8 % 4
                    # == 0), so each sem lane sees a single queue.
                    if BIGGATHER:
                        for b in range(NBANKS):
                            nc.gpsimd.dma_gather(
                                out_ap=msgs[b][:],
                                in_ap=xbf[b * BANK : (b + 1) * BANK, :],
                                idxs_ap=idx_all[:, g, b, :],
                                num_idxs=GIDX,
                                num_idxs_reg=GIDX,
                                elem_size=D,
                                single_packet=False,
                                queue_num=b % NQUEUES,
                            )
                    else:
                      for i in range(GROUP):
                        for b in range(NBANKS):
                            k = (g * NBANKS + b) * GROUP + i
                            nc.gpsimd.reg_load(
                                cnt_reg, cnt_sb[:1, k : k + 1]
                            )
                            nc.gpsimd.dma_gather(
                                out_ap=msgs[b][:, i * KB : (i + 1) * KB, :],
                                in_ap=xbf[b * BANK : (b + 1) * BANK, :],
                                idxs_ap=idx_all[
                                    :, g, b, i * TIW : (i + 1) * TIW
                                ],
                                num_idxs=KB * P,
                                num_idxs_reg=cnt_reg,
                                elem_size=D,
                                single_packet=True,
                                queue_num=b % NQUEUES,
                            )

                # root path: pre-transposed x tiles for the group
                xt_sb = xt_pool.tile([P, GROUP, D], f32, tag="xt")
                nc.sync.dma_start(xt_sb[:], xt[g])

                out_sb = out_pool.tile([P, GROUP, D], f32, tag="out")

                for i in range(GROUP):
                    t = g * GROUP + i
                    # weighted one-hots for all chunk slots of the tile in
                    # two batched tensor_tensor ops (TT never enters the
                    # DVE 2-port perf mode, so unlike tensor_scalar it does
                    # not lock GPSIMD's SWDGE descriptor generation out of
                    # SBUF -> the gathers overlap with one-hot building)
                    if "onehot" not in ablate_set:
                        oh_all = oh_pool.tile([P, KT, P], bf16, tag="ohall")
                        dst_b = (
                            dst_all[:, t, :].unsqueeze(2).broadcast_to([P, KT, P])
                        )
                        nc.vector.tensor_tensor(
                            out=oh_all[:],
                            in0=iota_rep[:],
                            in1=dst_b,
                            op=mybir.AluOpType.is_equal,
                        )
                    # aggregation: meanT[f, n] accumulated over chunk slots
                    ps = ps_sum_pool.tile([P, P], f32)
                    for s in range(KT):
                        b, j = divmod(s, KB)
                        rhs_ap = (
                            oh_all[:, s, :]
                            if "onehot" not in ablate_set
                            else iota_g[:]
                        )
                        if "chunkmm" not in ablate_set:
                            lhs_ap = (
                                msgs[b][:, i * KB + j, :]
                                if "gather" not in ablate_set
                                else iota_g[:]
                            )
                            nc.tensor.matmul(
                                out=ps[:],
                                lhsT=lhs_ap,
                                rhs=rhs_ap,
                                start=(s == 0),
                                stop=(s == KT - 1),
                            )
                    if "chunkmm" in ablate_set:
                        nc.tensor.matmul(
                            out=ps[:], lhsT=iota_g[:], rhs=iota_g[:],
                            start=True, stop=True,
                        )
                    sumT_sb = mean_pool.tile([P, P], f32, tag="sumT")
                    nc.scalar.copy(sumT_sb[:], ps[:])

                    # mean path: (sumT.T @ W_l) * (1/deg) per row
                    ps_a = ps_out_pool.tile([P, P], f32, tag="ps_a")
                    nc.tensor.matmul(
                        out=ps_a[:], lhsT=sumT_sb[:], rhs=wl_sb[:],
                        start=True, stop=True,
                    )
                    out_l = eg_pool.tile([P, P], f32, tag="out_l")
                    nc.vector.tensor_tensor(
                        out=out_l[:],
                        in0=ps_a[:],
                        in1=rcol_sb[:, t : t + 1].broadcast_to([P, D]),
                        op=mybir.AluOpType.mult,
                    )
                    # root path + bias, then combine
                    po = ps_out_pool.tile([P, P], f32, tag="ps_b")
                    nc.tensor.matmul(
                        out=po[:], lhsT=xt_sb[:, i, :], rhs=wr_sb[:],
                        start=True, stop=False,
                    )
                    nc.tensor.matmul(
                        out=po[:], lhsT=ones1[:], rhs=bl_sb[:],
                        start=False, stop=True,
                    )
                    final = eg_pool.tile([P, P], f32, tag="final")
                    nc.vector.tensor_tensor(
                        out=final[:], in0=out_l[:], in1=po[:],
                        op=mybir.AluOpType.add,
                    )

                    # row-wise L2 normalize: out / max(||out||, 1e-12)
                    sq = eg_pool.tile([P, P], f32, tag="sq")
                    ss = eg_pool.tile([P, 1], f32, tag="ss")
                    nc.scalar.activation(
                        sq[:],
                        final[:],
                        mybir.ActivationFunctionType.Square,
                        accum_out=ss[:],
                    )
                    nrm = eg_pool.tile([P, 1], f32, tag="nrm")
                    nc.scalar.activation(
                        nrm[:],
                        ss[:],
                        mybir.ActivationFunctionType.Sqrt,
                        bias=eps[:, :1],
                    )
                    rn = eg_pool.tile([P, 1], f32, tag="rn")
                    nc.vector.reciprocal(rn[:], nrm[:])
                    nc.vector.tensor_tensor(
                        out=out_sb[:, i, :],
                        in0=final[:],
                        in1=rn[:, :1].broadcast_to([P, D]),
                        op=mybir.AluOpType.mult,
                    )
                nc.sync.dma_start(out[g], out_sb[:])

    nc.compile()
    return nc


def _prepare(x, edge_index):
    """Host-side sharding: sort by dst, group per (tile, bank), pack chunks."""
    src = np.ascontiguousarray(edge_index[0]).astype(np.int64)
    dst = np.ascontiguousarray(edge_index[1]).astype(np.int64)

    cnt = np.bincount(dst, minlength=N_NODES)
    w_node = (1.0 / np.maximum(cnt, 1)).astype(np.float32)

    order = np.argsort(dst, kind="stable")
    src_s = src[order]
    dst_s = dst[order]

    # per-core edge ranges and per (core,tile,bank) grouping
    per_core = []
    KB = 1
    for c in range(N_CORES):
        base = c * NODES_PER_CORE
        lo = np.searchsorted(dst_s, base)
        hi = np.searchsorted(dst_s, base + NODES_PER_CORE)
        s_c = src_s[lo:hi]
        d_c = dst_s[lo:hi] - base
        t_c = d_c // P
        b_c = s_c // BANK
        key = (t_c * NBANKS + b_c).astype(np.int64)
        ordc = np.argsort(key, kind="stable")
        s_c, d_c, key = s_c[ordc], d_c[ordc], key[ordc]
        counts = np.bincount(key, minlength=TILES_PER_CORE * NBANKS)
        KB = max(KB, int(np.ceil(counts.max() / P)))
        per_core.append((s_c, d_c, counts))

    KT = NBANKS * KB
    NIDX = KB * P
    IW = NIDX // 16

    import ml_dtypes

    gidx = np.zeros((N_CORES, P, TILES_PER_CORE, NBANKS, IW), np.int16)
    dstrel = np.full((N_CORES, P, TILES_PER_CORE, KT), -1.0, ml_dtypes.bfloat16)

    # per-node 1/max(deg,1) as [core][lane, tile] columns
    wg_full = np.zeros(N_NODES, np.float32)
    wg_full[:] = w_node
    rcol = np.ones((N_CORES, P, TILES_PER_CORE), np.float32)
    for c in range(N_CORES):
        idx = (
            c * NODES_PER_CORE
            + (np.arange(TILES_PER_CORE) * P)[None, :]
            + np.arange(P)[:, None]
        )
        valid = idx < N_NODES
        rcol[c][valid] = wg_full[idx[valid]]

    prow = np.arange(P) % 16
    scol = np.arange(IW) * 16
    for c in range(N_CORES):
        s_c, d_c, counts = per_core[c]
        starts = np.concatenate([[0], np.cumsum(counts)])
        for t in range(TILES_PER_CORE):
            for b in range(NBANKS):
                gidx_g = t * NBANKS + b
                n = counts[gidx_g]
                if n == 0:
                    continue
                lo = starts[gidx_g]
                sv = s_c[lo : lo + n] - b * BANK
                dv = (d_c[lo : lo + n] - t * P).astype(np.float32)
                i_pad = (
                    np.zeros(NIDX, np.int16)
                    if BIGGATHER
                    else np.full(NIDX, -1, np.int16)
                )
                i_pad[:n] = sv.astype(np.int16)
                d_pad = np.full(NIDX, -1.0, np.float32)
                d_pad[:n] = dv
                # idx position i lives at [i % 16, i // 16], replicated %16
                gidx[c, :, t, b, :] = i_pad[scol[None, :] + prow[:, None]]
                # chunk slot s=b*KB+j, lane p <- edge j*128+p
                dstrel[c, :, t, b * KB : (b + 1) * KB] = d_pad.reshape(KB, P).T

    # per-(group, bank, tile-in-group) real index counts, [NG*NB*GROUP]
    cnts = np.zeros((N_CORES, NG, NBANKS, GROUP), np.int32)
    for c in range(N_CORES):
        counts = per_core[c][2].reshape(TILES_PER_CORE, NBANKS)
        cnts[c] = counts.reshape(NG, GROUP, NBANKS).transpose(0, 2, 1)
    cnts = cnts.reshape(N_CORES, 1, NG * NBANKS * GROUP)

    # regroup gather indices: [P, T, NB, IW] -> [P, NG, NB, GROUP*IW]
    # (concatenating a group's per-tile index blocks along the column axis
    #  is exactly the int16 16-row wrap of the concatenated index list,
    #  since each block holds a multiple of 16 indices)
    gidx_g = (
        gidx.reshape(N_CORES, P, NG, GROUP, NBANKS, IW)
        .transpose(0, 1, 2, 4, 3, 5)
        .reshape(N_CORES, P, NG, NBANKS, GROUP * IW)
    )
    gidx_g = np.ascontiguousarray(gidx_g)

    return gidx_g, dstrel, rcol, cnts, KB


def unswizzle_out(arr):
    """[NG, P, GROUP*D] device layout -> [NODES_PER_CORE, D]."""
    return (
        arr.reshape(NG, P, GROUP, D)
        .transpose(0, 2, 1, 3)
        .reshape(NODE_PAD, D)[:NODES_PER_CORE]
    )


def kernel(x, edge_index, W_l, b_l, W_r):
    import ml_dtypes
    from concourse.bass_utils import run_bass_kernel_spmd

    x = np.ascontiguousarray(np.asarray(x, dtype=np.float32))
    W_l = np.ascontiguousarray(np.asarray(W_l, dtype=np.float32))
    W_r = np.ascontiguousarray(np.asarray(W_r, dtype=np.float32))
    b_l = np.ascontiguousarray(np.asarray(b_l, dtype=np.float32)).reshape(1, D)

    gidx, dstrel, rcol, cnts, KB = _prepare(x, np.asarray(edge_index))

    xpad = np.zeros((X_PAD_ROWS, D), np.float32)
    xpad[:N_NODES] = x
    xbf = xpad.astype(ml_dtypes.bfloat16)

    if KB not in _program_cache:
        _program_cache[KB] = _build_program(KB)
    nc = _program_cache[KB]

    in_maps = []
    for c in range(N_CORES):
        base = c * NODES_PER_CORE
        slab = xpad[base : base + NODE_PAD]  # [12544, 128]
        xt_c = np.ascontiguousarray(
            slab.reshape(NG, GROUP, P, D).transpose(0, 3, 1, 2).reshape(
                NG, P, GROUP * D
            )
        )
        in_maps.append(
            {
                "xbf": xbf,
                "xt": xt_c,
                "gidx": gidx[c],
                "cnt": cnts[c],
                "dstrel": dstrel[c],
                "rcol": rcol[c],
                "wl": W_l,
                "wr": W_r,
                "bl": b_l,
            }
        )

    LAST["nc"] = nc
    LAST["in_maps"] = in_maps
    r = run_bass_kernel_spmd(nc, in_maps, list(range(N_CORES)), trace=TRACE)
    LAST["exec_time_ns"] = r.exec_time_ns
    res = r.results
    out = np.concatenate(
        [unswizzle_out(res[c]["out"]) for c in range(N_CORES)], axis=0
    )
    return out
